# revision 32
# baseline (speedup 1.0000x reference)
"""Multi-head causal attention (GQA + QK-RMSNorm + RoPE) for Trainium2, 8 cores.

Sharding: 8 cores = 2 batches x 4 head-groups (tensor parallel over heads).
Each core handles one batch and 8 Q heads + 2 KV heads:
  - QKV projection for its head slice (fp16 matmuls, fp32 psum)
  - QK RMSNorm + RoPE (gamma folded into host-side cos/sin tables)
  - causal attention in k-major (transposed-scores) layout:
      sT[k, q] = kT.T @ qT ; p = exp(sT/8) ; ctx = pT.T @ [v | 1]
    (the ones column yields the softmax denominator for free)
  - output projection against its w_out column slice -> partial [S, D]
Host sums the 4 head-group partials per batch.
"""
import sys
import numpy as np
from contextlib import ExitStack

if '/opt/trn_rl_repo' not in sys.path:
    sys.path.insert(0, '/opt/trn_rl_repo')

import concourse.bacc as bacc
import concourse.tile as tile
import concourse.mybir as mybir
from concourse.bass_utils import run_bass_kernel_spmd

dt = mybir.dt
AF = mybir.ActivationFunctionType
AX = mybir.AxisListType
ALU = mybir.AluOpType

HEAD_DIM = 64
NUM_Q_HEADS = 32
NUM_KV_HEADS = 8
ROPE_FREQ = 10000.0
EPS = 1e-6

B, S, D = 2, 2048, 2048
QH = 8            # q heads per core
KVH = 2           # kv heads per core
N_CORES = 8
ST = S // 128      # 16 token tiles of 128
NJ = S // 512      # 4 big q blocks of 512

F16 = dt.float16
F32 = dt.float32


def _build(repeat=1):
    nc = bacc.Bacc("TRN2", target_bir_lowering=False, debug=False,
                   num_devices=N_CORES)

    xT = nc.dram_tensor("xT", [D, S], F16, kind="ExternalInput").ap()
    wqkvT = nc.dram_tensor("wqkvT", [D, (QH + 2 * KVH) * HEAD_DIM], F16,
                           kind="ExternalInput").ap()          # cols: 512 q | 128 k | 128 v
    woutT = nc.dram_tensor("woutT", [QH * HEAD_DIM, D], F16,
                           kind="ExternalInput").ap()
    cos_q = nc.dram_tensor("cos_q", [S, HEAD_DIM], F16, kind="ExternalInput").ap()
    sin_q = nc.dram_tensor("sin_q", [S, HEAD_DIM], F16, kind="ExternalInput").ap()
    cos_k = nc.dram_tensor("cos_k", [S, HEAD_DIM], F16, kind="ExternalInput").ap()
    sin_k = nc.dram_tensor("sin_k", [S, HEAD_DIM], F16, kind="ExternalInput").ap()
    mask_i = nc.dram_tensor("mask_i", [128, 128], F16, kind="ExternalInput").ap()
    out = nc.dram_tensor("out", [S, D], F16, kind="ExternalOutput").ap()

    with tile.TileContext(nc) as tc, ExitStack() as ctx:
        perm = ctx.enter_context(tc.tile_pool(name="perm", bufs=1))

        # ---- persistent tiles ----
        w_ch = [perm.tile([128, 768], F16, tag=f"w{dc}", name=f"w{dc}") for dc in range(16)]
        wo_ch = [perm.tile([128, D], F16, tag=f"wo{p}", name=f"wo{p}") for p in range(4)]
        tabs = {}
        for nm in ("cq", "sq", "ck", "sk"):
            tabs[nm] = perm.tile([128, ST, HEAD_DIM], F16, tag=f"tab{nm}", name=f"tab{nm}")
        tab_srcs = {"cq": cos_q, "sq": sin_q, "ck": cos_k, "sk": sin_k}
        tmask = perm.tile([128, 128], F16, tag="mask")
        teps = perm.tile([128, 1], F32, tag="eps")
        nc.vector.memset(teps[:], EPS)
        tzero = perm.tile([1, 512], F16, tag="tzero")
        nc.vector.memset(tzero[:], 0.0)

        qT = [perm.tile([128, S], F16, tag=f"qT{p}", name=f"qT{p}") for p in range(4)]
        kT = perm.tile([128, S], F16, tag="kT", name="kT")
        vext = [[perm.tile([128, HEAD_DIM + 1], F16, tag=f"v{g}_{i}", name=f"v{g}_{i}")
                 for i in range(ST)] for g in range(KVH)]
        for g in range(KVH):
            for i in range(ST):
                nc.vector.memset(vext[g][i][:, HEAD_DIM:HEAD_DIM + 1], 1.0)
        ctxT = [perm.tile([128, S], F16, tag=f"ctxT{p}", name=f"ctxT{p}") for p in range(4)]

        rep_ctx = tc.For_i(0, repeat, 1) if repeat > 1 else None
        if rep_ctx is not None:
            ctx.enter_context(rep_ctx)

        # ---- shared pools across all phases (no phase barriers) ----
        psum = ctx.enter_context(tc.tile_pool(name="psum", bufs=2, space="PSUM"))
        p1sb = ctx.enter_context(tc.tile_pool(name="p1sb", bufs=4))
        p1st = ctx.enter_context(tc.tile_pool(name="p1st", bufs=2))
        ptpool = ctx.enter_context(tc.tile_pool(name="ptpool", bufs=6))
        cnpool = ctx.enter_context(tc.tile_pool(name="cnpool", bufs=3))
        rcpool = ctx.enter_context(tc.tile_pool(name="rcpool", bufs=8))
        osb = ctx.enter_context(tc.tile_pool(name="osb", bufs=6))

        # ================= Phase 1: QKV + norm + rope + transposes ============
        if True:
            def p1_transposes(i, qr):
                # XBAR DMA transposes (DMA engines are idle here). Host packs
                # q heads in pair order (p, p+4) so partition halves line up
                # with kT's natural 2-head layout (no k duplication needed).
                for p in range(4):
                    nc.sync.dma_start_transpose(
                        qT[p][:, 128 * i:128 * (i + 1)],
                        qr[:, 128 * p:128 * (p + 1)])
                nc.sync.dma_start_transpose(
                    kT[:, 128 * i:128 * (i + 1)], qr[:, 512:640])

            xt_tiles = {}

            def load_xt(i2):
                t = p1sb.tile([128, 16, 256], F16, tag="xt", name="xt")
                nc.sync.dma_start(
                    t[:], xT[:, 256 * i2:256 * (i2 + 1)]
                    .rearrange("(n p) s -> p n s", p=128))
                xt_tiles[i2] = t

            load_xt(0)
            for dc in range(16):
                nc.sync.dma_start(w_ch[dc][:], wqkvT[128 * dc:128 * (dc + 1), :])
            load_xt(1)
            for nm in ("cq", "sq", "ck", "sk"):
                nc.sync.dma_start(tabs[nm][:],
                                  tab_srcs[nm].rearrange("(n p) d -> p n d", p=128))
            nc.sync.dma_start(tmask[:], mask_i[:])
        if True:
            # Unified filler-work queue: out-proj jobs and p1 (QKV) quanta are
            # drained into the attention c-loops, filling PE slots while ACT
            # streams exps.
            work_q = []

            def pop_work(n=1):
                for _ in range(n):
                    if work_q:
                        work_q.pop(0)()

            def emit_po(i, dch):
                po = psum.tile([128, 512], F32, tag="tp", name="po")
                for p in range(4):
                    nc.tensor.matmul(
                        po[:], ctxT[p][:, 128 * i:128 * (i + 1)],
                        wo_ch[p][:, 512 * dch:512 * (dch + 1)],
                        start=(p == 0), stop=(p == 3))
                ob = osb.tile([128, 512], F16, tag="ob", name="ob")
                nc.vector.tensor_copy(ob[:], po[:])
                nc.sync.dma_start(
                    out[128 * i:128 * (i + 1), 512 * dch:512 * (dch + 1)],
                    ob[:])

            def emit_p1(i):
                """QKV + norm/rope for token chunk i (emitted inline)."""
                pq = psum.tile([128, 512], F32, tag="pq", name="pq",
                                padded_shape=[128, 1024])
                pkv = psum.tile([128, 256], F32, tag="pkv", name="pkv",
                                padded_shape=[128, 260])
                xt = xt_tiles[i // 2]
                for dc in range(16):
                    xs = xt[:, dc, 128 * (i % 2):128 * (i % 2 + 1)]
                    nc.tensor.matmul(pq[:], xs, w_ch[dc][:, 0:512],
                                     start=(dc == 0), stop=(dc == 15))
                for dc in range(16):
                    xs = xt[:, dc, 128 * (i % 2):128 * (i % 2 + 1)]
                    nc.tensor.matmul(pkv[:], xs, w_ch[dc][:, 512:768],
                                     start=(dc == 0), stop=(dc == 15))
                qsb = p1sb.tile([128, 640], F16, tag="qsb")
                nc.scalar.copy(qsb[:, 0:512], pq[:])
                nc.scalar.copy(qsb[:, 512:640], pkv[:, 0:128])
                # v chunks straight out to vext tiles
                nc.scalar.copy(vext[0][i][:, 0:HEAD_DIM], pkv[:, 128:192])
                nc.scalar.copy(vext[1][i][:, 0:HEAD_DIM], pkv[:, 192:256])

                # --- RMSNorm (per 64) + RoPE on q (8 heads) and k (2 heads)
                sqt = p1sb.tile([128, 640], F16, tag="sqt")
                nc.scalar.square(sqt[:], qsb[:])
                ssum = p1st.tile([128, 10], F32, tag="ssum")
                nc.vector.tensor_reduce(
                    ssum[:], sqt[:].rearrange("p (h d) -> p h d", h=10),
                    axis=AX.X, op=ALU.add)
                lnv = p1st.tile([128, 10], F32, tag="std")
                nc.scalar.activation(lnv[:], ssum[:], AF.Ln,
                                     bias=teps[:], scale=1.0 / HEAD_DIM)
                rstd = p1st.tile([128, 10], F32, tag="rstd")
                nc.scalar.activation(rstd[:], lnv[:], AF.Exp, scale=-0.5)
                qn = p1sb.tile([128, 640], F16, tag="qn")
                nc.vector.tensor_mul(
                    qn[:].rearrange("p (h d) -> p h d", h=10),
                    qsb[:].rearrange("p (h d) -> p h d", h=10),
                    rstd[:].unsqueeze(2).broadcast_to([128, 10, HEAD_DIM]))
                qr = p1sb.tile([128, 640], F16, tag="qr")
                t2 = p1sb.tile([128, 640], F16, tag="t2")
                for part, nh, c_t, s_t in (("q", QH, "cq", "sq"),
                                           ("k", KVH, "ck", "sk")):
                    off = 0 if part == "q" else 512
                    qn3 = qn[:, off:off + 64 * nh].rearrange(
                        "p (h d) -> p h d", h=nh)
                    qn4 = qn[:, off:off + 64 * nh].rearrange(
                        "p (h two x) -> p h two x", h=nh, two=2)
                    cosb = tabs[c_t][:, i, :].unsqueeze(1) \
                        .broadcast_to([128, nh, HEAD_DIM])
                    sin4 = tabs[s_t][:, i, :].unsqueeze(1) \
                        .broadcast_to([128, nh, HEAD_DIM]) \
                        .rearrange("p h (two x) -> p h two x", two=2)
                    t2v = t2[:, off:off + 64 * nh].rearrange(
                        "p (h two x) -> p h two x", h=nh, two=2)
                    nc.vector.tensor_mul(t2v[:, :, 0, :], qn4[:, :, 1, :],
                                         sin4[:, :, 0, :])
                    nc.vector.tensor_mul(t2v[:, :, 1, :], qn4[:, :, 0, :],
                                         sin4[:, :, 1, :])
                    qr3 = qr[:, off:off + 64 * nh].rearrange(
                        "p (h d) -> p h d", h=nh)
                    nc.vector.tensor_mul(qr3, qn3, cosb)
                nc.vector.tensor_add(qr[:], qr[:], t2[:])
                p1_transposes(i, qr)

            def attn_J(J):
                ctxn = [cnpool.tile([128, 512], F16, tag=f"cn{jj}", name=f"cn{jj}")
                        for jj in range(4)]
                for hp in range(4):          # q heads (hp, hp+4); half hh -> kv hh
                    cbank = [psum.tile([128, 260], F32, tag="pkv", name=f"cb{w}")
                             for w in (0, 1)]
                    def epi(jj):
                        w, loc = jj // 2, jj % 2
                        cb2 = cbank[w][:, 130 * loc:130 * (loc + 1)].rearrange(
                            "p (h e) -> p h e", h=2)
                        rc = rcpool.tile([128, 2], F32, tag="rc", name="rc")
                        nc.vector.reciprocal(rc[:], cb2[:, :, 64:65].squeeze(2))
                        nc.vector.tensor_mul(
                            ctxn[jj][:, 128 * hp:128 * (hp + 1)].rearrange(
                                "p (h d) -> p h d", h=2),
                            cb2[:, :, 0:64],
                            rc[:].unsqueeze(2).broadcast_to([128, 2, HEAD_DIM]))
                        nc.sync.dma_start_transpose(
                            ctxT[hp][:, 512 * J + 128 * jj:512 * J + 128 * (jj + 1)],
                            ctxn[jj][:, 128 * hp:128 * (hp + 1)])

                    def ctx_mms(pt, jj0, c):
                        for jj in range(jj0, 4):
                            w, loc = jj // 2, jj % 2
                            for hh in (0, 1):
                                o = 130 * loc + 65 * hh
                                nc.tensor.matmul(
                                    cbank[w][:, o:o + 65],
                                    pt[:, 512 * hh + 128 * jj:512 * hh + 128 * (jj + 1)],
                                    vext[hh][c][:],
                                    start=(c == 0 and loc == 0 and hh == 0),
                                    stop=(c == 4 * J + jj and jj % 2 == 1 and hh == 1),
                                    skip_group_check=True)

                    def drain_ctx():
                        pt_, jj0_, c_ = pending.pop(0)
                        ctx_mms(pt_, jj0_, c_)
                        if c_ == 4 * J + 1:    # bank 0 (jj 0,1) is complete
                            epi(0)
                            epi(1)

                    # ctx lags scores by 2 chunks so the in-order PE queue
                    # never waits on an in-flight exp
                    pending = []
                    for c in range(4 * J + 4):
                        jj0 = max(0, c - 4 * J)
                        qlo = 128 * jj0
                        sT2 = psum.tile([128, 1024], F32, tag="pq", name="sT2")
                        for hh in (0, 1):
                            nc.tensor.matmul(
                                sT2[:, 512 * hh + qlo:512 * (hh + 1)],
                                kT[64 * hh:64 * hh + 64, 128 * c:128 * (c + 1)],
                                qT[hp][64 * hh:64 * hh + 64,
                                       512 * J + qlo:512 * (J + 1)],
                                start=True, stop=True)
                        if len(pending) >= 2:
                            drain_ctx()
                        pop_work(1)
                        pt = ptpool.tile([128, 1024], F16, tag="pt", name="pt")
                        ptv = pt[:].rearrange("p (h x) -> p h x", h=2)[:, :, 128 * jj0:512]
                        sTv = sT2[:].rearrange("p (h x) -> p h x", h=2)[:, :, 128 * jj0:512]
                        nc.scalar.activation(ptv, sTv, AF.Exp, scale=0.125)
                        if c >= 4 * J:      # diagonal: triangular mask, both heads
                            dv = pt[:].rearrange("p (h x) -> p h x", h=2)[
                                :, :, 128 * jj0:128 * (jj0 + 1)]
                            nc.vector.tensor_mul(
                                dv, dv,
                                tmask[:].unsqueeze(1).broadcast_to([128, 2, 128]))
                        pending.append((pt, jj0, c))
                    drain_ctx()
                    pop_work(1)          # filler under the last exp's latency
                    drain_ctx()
                    for jj in (2, 3):
                        epi(jj)
                    pop_work(1)
                # defer out-proj; drained inside the next J's c-loop
                for i in range(4 * J, 4 * J + 4):
                    for dch in range(4):
                        work_q.append(lambda i=i, dch=dch: emit_po(i, dch))


            for i in range(ST):
                i2 = i // 2
                if i % 2 == 0 and i2 + 1 < ST // 2 and i2 + 1 not in xt_tiles:
                    load_xt(i2 + 1)
                emit_p1(i)
            for p in range(4):
                nc.sync.dma_start(wo_ch[p][:], woutT[128 * p:128 * (p + 1), :])
            for J in range(NJ):
                attn_J(J)
            while work_q:
                pop_work(1)

        # ================= Phase 2: attention (+ interleaved out-proj) ========
        # head pairs share one 1024-wide scores/exp tile; ctx accumulators for
        # (jj, head-half) pack into two [128, 260] psum banks per pair.
    nc.compile()
    return nc


_NC = {}


def _get_nc(repeat=1):
    if repeat not in _NC:
        _NC[repeat] = _build(repeat)
    return _NC[repeat]


_RUNNER = {}


def _get_runner(repeat=1):
    """Build (once) a jitted 8-core sharded callable around the bass program.

    Slim replica of bass2jax.run_bass_via_pjrt's multi-core path, kept
    reusable so repeated invocations skip retracing/recompilation.
    """
    if repeat in _RUNNER:
        return _RUNNER[repeat]
    import jax
    from jax.sharding import Mesh, PartitionSpec
    from jax.experimental.shard_map import shard_map
    from concourse import bass2jax
    from concourse import mybir as _mybir

    nc = _get_nc(repeat)
    bass2jax.install_neuronx_cc_hook()

    partition_name = nc.partition_id_tensor.name if nc.partition_id_tensor else None
    in_names, out_names, out_avals, zero_outs = [], [], [], []
    for alloc in nc.m.functions[0].allocations:
        if not isinstance(alloc, _mybir.MemoryLocationSet):
            continue
        name = alloc.memorylocations[0].name
        if alloc.kind == "ExternalInput":
            if name != partition_name:
                in_names.append(name)
        elif alloc.kind == "ExternalOutput":
            shape = tuple(alloc.tensor_shape)
            np_dt = _mybir.dt.np(alloc.dtype)
            out_names.append(name)
            out_avals.append(jax.core.ShapedArray(shape, np_dt))
            zero_outs.append(np.zeros(shape, np_dt))
    n_params = len(in_names)
    all_in_names = list(in_names) + list(out_names)
    if partition_name is not None:
        all_in_names.append(partition_name)

    def _body(*args):
        operands = list(args)
        if partition_name is not None:
            operands.append(bass2jax.partition_id_tensor())
        outs = bass2jax._bass_exec_p.bind(
            *operands,
            out_avals=tuple(out_avals),
            in_names=tuple(all_in_names),
            out_names=tuple(out_names),
            lowering_input_output_aliases=(),
            sim_require_finite=True,
            sim_require_nnan=True,
            nc=nc,
        )
        return tuple(outs)

    devices = jax.devices()[:N_CORES]
    mesh = Mesh(np.asarray(devices), ("core",))
    in_specs = (PartitionSpec("core"),) * (n_params + len(out_names))
    out_specs = (PartitionSpec("core"),) * len(out_names)
    sharded = jax.jit(shard_map(_body, mesh=mesh, in_specs=in_specs,
                                out_specs=out_specs, check_rep=False),
                      keep_unused=True)

    concat_zeros = [np.zeros((N_CORES * z.shape[0], *z.shape[1:]), z.dtype)
                    for z in zero_outs]

    _dev_cache = {}

    def run(in_maps, iters=1, time_list=None, fetch=True):
        import time as _time
        from jax.sharding import NamedSharding
        shard = NamedSharding(mesh, PartitionSpec("core"))
        key = id(in_maps)
        if key not in _dev_cache:
            per_core = [[np.asarray(m[nm]) for nm in in_names] for m in in_maps]
            concat_in = [np.concatenate([per_core[c][i] for c in range(N_CORES)],
                                        axis=0) for i in range(n_params)]
            dev_in = [jax.device_put(a, shard) for a in concat_in]
            dev_zero = [jax.device_put(z, shard) for z in concat_zeros]
            jax.block_until_ready(dev_in)
            _dev_cache.clear()
            _dev_cache[key] = (dev_in, dev_zero)
        dev_in, dev_zero = _dev_cache[key]
        out_arrs = None
        if iters <= 1:
            out_arrs = sharded(*dev_in, *dev_zero)
            jax.block_until_ready(out_arrs)
        else:
            # async batch: submit all, block once; caller computes slope
            sharded(*dev_in, *dev_zero)  # warm
            t0 = _time.perf_counter()
            for _ in range(iters):
                out_arrs = sharded(*dev_in, *dev_zero)
            jax.block_until_ready(out_arrs)
            if time_list is not None:
                time_list.append(_time.perf_counter() - t0)
        if not fetch:
            del out_arrs
            return None
        return [
            {nm: np.asarray(out_arrs[i]).reshape(N_CORES, *out_avals[i].shape)[c]
             for i, nm in enumerate(out_names)}
            for c in range(N_CORES)
        ]

    _RUNNER[repeat] = run
    return run


def _host_tables(q_gamma, k_gamma):
    pos = np.arange(S, dtype=np.float32)
    inv = 1.0 / (ROPE_FREQ ** (np.arange(0, HEAD_DIM, 2, dtype=np.float32)
                               / HEAD_DIM))
    fr = pos[:, None] * inv[None, :]
    emb = np.concatenate([fr, fr], axis=-1)
    cos = np.cos(emb).astype(np.float32)
    sin = np.sin(emb).astype(np.float32)
    outs = []
    for gamma in (q_gamma, k_gamma):
        g = gamma.astype(np.float32)
        cos_g = cos * g[None, :]
        sin_eff = np.concatenate([-sin[:, :32] * g[None, 32:],
                                  sin[:, 32:] * g[None, :32]], axis=-1)
        outs += [cos_g.astype(np.float16), sin_eff.astype(np.float16)]
    return outs  # cos_q, sin_q, cos_k, sin_k


def _make_in_maps(x, w_qkv, w_out, q_gamma, k_gamma):
    cos_q, sin_q, cos_k, sin_k = _host_tables(q_gamma, k_gamma)
    mask = (np.arange(128)[None, :] >= np.arange(128)[:, None]).astype(np.float16)
    # q heads packed in pairs (p, p+4) so a pair's partition halves map to
    # kv heads (0, 1) = kT's natural layout
    qperm = [0, 4, 1, 5, 2, 6, 3, 7]

    in_maps = []
    for core in range(N_CORES):
        b, g = core // 4, core % 4
        xT = np.ascontiguousarray(x[b].T).astype(np.float16)
        wq = w_qkv[512 * g:512 * (g + 1)]                      # 8 q heads
        wq = wq.reshape(8, HEAD_DIM, D)[qperm].reshape(512, D)
        wk = w_qkv[2048 + 128 * g:2048 + 128 * (g + 1)]        # 2 k heads
        wv = w_qkv[2560 + 128 * g:2560 + 128 * (g + 1)]        # 2 v heads
        wqkvT = np.ascontiguousarray(
            np.concatenate([wq, wk, wv], axis=0).T).astype(np.float16)
        woutT = w_out[:, 512 * g:512 * (g + 1)].T              # [512, D]
        woutT = np.ascontiguousarray(
            woutT.reshape(8, HEAD_DIM, D)[qperm].reshape(512, D)
        ).astype(np.float16)
        in_maps.append({
            "xT": xT, "wqkvT": wqkvT, "woutT": woutT,
            "cos_q": cos_q, "sin_q": sin_q, "cos_k": cos_k, "sin_k": sin_k,
            "mask_i": mask,
        })
    return in_maps


def kernel(x, w_qkv, w_out, q_gamma, k_gamma):
    x = np.asarray(x)
    w_qkv = np.asarray(w_qkv)
    w_out = np.asarray(w_out)
    q_gamma = np.asarray(q_gamma)
    k_gamma = np.asarray(k_gamma)
    in_maps = _make_in_maps(x, w_qkv, w_out, q_gamma, k_gamma)
    results = _get_runner()(in_maps)
    parts = [results[c]["out"] for c in range(N_CORES)]
    out = np.empty((B, S, D), dtype=np.float32)
    for b in range(B):
        acc = parts[4 * b].astype(np.float32)
        for g in range(1, 4):
            acc += parts[4 * b + g].astype(np.float32)
        out[b] = acc
    return out



# revision 36
# speedup vs baseline: 1.0700x; 1.0700x over previous
"""Multi-head causal attention (GQA + QK-RMSNorm + RoPE) for Trainium2, 8 cores.

Sharding: 8 cores = 2 batches x 4 head-groups (tensor parallel over heads).
Each core handles one batch and 8 Q heads + 2 KV heads:
  - QKV projection for its head slice (fp16 matmuls, fp32 psum);
    RMSNorm rsqrt computed as exp(-0.5*ln(v)) so the whole kernel stays on
    one ACT table set (natural_log_exp_and_others) - no table reloads
  - QK RMSNorm + RoPE (gamma folded into host-side fp16 cos/sin tables)
  - q/k transposed to dim-major via XBAR DMA transposes (PE stays free);
    q heads packed in pairs (p, p+4) host-side so a pair's partition halves
    line up with kT's natural 2-head layout (no k duplication)
  - causal attention in k-major (transposed-scores) layout:
      sT[k, q] = kT.T @ qT ; p = exp(sT/8) ; ctx = pT.T @ [v | 1]
    (the ones column yields the softmax denominator for free); scores and
    exp truncated below the causal diagonal; ctx matmuls trail scores by 2
    chunks so the in-order PE queue never waits on an in-flight exp
  - output projection against its w_out column slice, interleaved into the
    attention c-loops as filler -> fp16 partial [S, D]
Host sums the 4 head-group partials per batch in fp32.
"""
import sys
import numpy as np
from contextlib import ExitStack

if '/opt/trn_rl_repo' not in sys.path:
    sys.path.insert(0, '/opt/trn_rl_repo')

import concourse.bacc as bacc
import concourse.tile as tile
import concourse.mybir as mybir
from concourse.bass_utils import run_bass_kernel_spmd

dt = mybir.dt
AF = mybir.ActivationFunctionType
AX = mybir.AxisListType
ALU = mybir.AluOpType

HEAD_DIM = 64
NUM_Q_HEADS = 32
NUM_KV_HEADS = 8
ROPE_FREQ = 10000.0
EPS = 1e-6

B, S, D = 2, 2048, 2048
QH = 8            # q heads per core
KVH = 2           # kv heads per core
N_CORES = 8
ST = S // 128      # 16 token tiles of 128
NJ = S // 512      # 4 big q blocks of 512

F16 = dt.float16
F32 = dt.float32


def _build(repeat=1):
    nc = bacc.Bacc("TRN2", target_bir_lowering=False, debug=False,
                   num_devices=N_CORES)

    xT = nc.dram_tensor("xT", [D, S], F16, kind="ExternalInput").ap()
    wqkvT = nc.dram_tensor("wqkvT", [D, (QH + 2 * KVH) * HEAD_DIM], F16,
                           kind="ExternalInput").ap()          # cols: 512 q | 128 k | 128 v
    woutT = nc.dram_tensor("woutT", [QH * HEAD_DIM, D], F16,
                           kind="ExternalInput").ap()
    cos_q = nc.dram_tensor("cos_q", [S, HEAD_DIM], F16, kind="ExternalInput").ap()
    sin_q = nc.dram_tensor("sin_q", [S, HEAD_DIM], F16, kind="ExternalInput").ap()
    cos_k = nc.dram_tensor("cos_k", [S, HEAD_DIM], F16, kind="ExternalInput").ap()
    sin_k = nc.dram_tensor("sin_k", [S, HEAD_DIM], F16, kind="ExternalInput").ap()
    mask_i = nc.dram_tensor("mask_i", [128, 128], F16, kind="ExternalInput").ap()
    out = nc.dram_tensor("out", [S, D], F16, kind="ExternalOutput").ap()

    with tile.TileContext(nc) as tc, ExitStack() as ctx:
        perm = ctx.enter_context(tc.tile_pool(name="perm", bufs=1))

        # ---- persistent tiles ----
        w_ch = [perm.tile([128, 768], F16, tag=f"w{dc}", name=f"w{dc}") for dc in range(16)]
        wo_ch = [perm.tile([128, D], F16, tag=f"wo{p}", name=f"wo{p}") for p in range(4)]
        tabs = {}
        for nm in ("cq", "sq", "ck", "sk"):
            tabs[nm] = perm.tile([128, ST, HEAD_DIM], F16, tag=f"tab{nm}", name=f"tab{nm}")
        tab_srcs = {"cq": cos_q, "sq": sin_q, "ck": cos_k, "sk": sin_k}
        tmask = perm.tile([128, 128], F16, tag="mask")
        teps = perm.tile([128, 1], F32, tag="eps")
        nc.vector.memset(teps[:], EPS)
        tzero = perm.tile([1, 512], F16, tag="tzero")
        nc.vector.memset(tzero[:], 0.0)

        qT = [perm.tile([128, S], F16, tag=f"qT{p}", name=f"qT{p}") for p in range(4)]
        kT = perm.tile([128, S], F16, tag="kT", name="kT")
        vext = [[perm.tile([128, HEAD_DIM + 1], F16, tag=f"v{g}_{i}", name=f"v{g}_{i}")
                 for i in range(ST)] for g in range(KVH)]
        for g in range(KVH):
            for i in range(ST):
                nc.vector.memset(vext[g][i][:, HEAD_DIM:HEAD_DIM + 1], 1.0)
        ctxT = [perm.tile([128, S], F16, tag=f"ctxT{p}", name=f"ctxT{p}") for p in range(4)]

        rep_ctx = tc.For_i(0, repeat, 1) if repeat > 1 else None
        if rep_ctx is not None:
            ctx.enter_context(rep_ctx)

        # ---- shared pools across all phases (no phase barriers) ----
        psum = ctx.enter_context(tc.tile_pool(name="psum", bufs=2, space="PSUM"))
        p1sb = ctx.enter_context(tc.tile_pool(name="p1sb", bufs=4))
        p1st = ctx.enter_context(tc.tile_pool(name="p1st", bufs=2))
        ptpool = ctx.enter_context(tc.tile_pool(name="ptpool", bufs=6))
        cnpool = ctx.enter_context(tc.tile_pool(name="cnpool", bufs=3))
        rcpool = ctx.enter_context(tc.tile_pool(name="rcpool", bufs=8))
        osb = ctx.enter_context(tc.tile_pool(name="osb", bufs=6))

        # ================= Phase 1: QKV + norm + rope + transposes ============
        if True:
            def p1_transposes(i, qr):
                # XBAR DMA transposes (DMA engines are idle here). Host packs
                # q heads in pair order (p, p+4) so partition halves line up
                # with kT's natural 2-head layout (no k duplication needed).
                for p in range(4):
                    nc.sync.dma_start_transpose(
                        qT[p][:, 128 * i:128 * (i + 1)],
                        qr[:, 128 * p:128 * (p + 1)])
                nc.sync.dma_start_transpose(
                    kT[:, 128 * i:128 * (i + 1)], qr[:, 512:640])

            xt_tiles = {}

            def load_xt(i2):
                t = p1sb.tile([128, 16, 256], F16, tag="xt", name="xt")
                nc.sync.dma_start(
                    t[:], xT[:, 256 * i2:256 * (i2 + 1)]
                    .rearrange("(n p) s -> p n s", p=128))
                xt_tiles[i2] = t

            load_xt(0)
            for dc in range(16):
                nc.sync.dma_start(w_ch[dc][:], wqkvT[128 * dc:128 * (dc + 1), :])
            load_xt(1)
            for nm in ("cq", "sq", "ck", "sk"):
                nc.sync.dma_start(tabs[nm][:],
                                  tab_srcs[nm].rearrange("(n p) d -> p n d", p=128))
            nc.sync.dma_start(tmask[:], mask_i[:])
        if True:
            # Unified filler-work queue: out-proj jobs and p1 (QKV) quanta are
            # drained into the attention c-loops, filling PE slots while ACT
            # streams exps.
            work_q = []

            def pop_work(n=1):
                for _ in range(n):
                    if work_q:
                        work_q.pop(0)()

            def emit_po(i, dch):
                po = psum.tile([128, 512], F32, tag="tp", name="po")
                for p in range(4):
                    nc.tensor.matmul(
                        po[:], ctxT[p][:, 128 * i:128 * (i + 1)],
                        wo_ch[p][:, 512 * dch:512 * (dch + 1)],
                        start=(p == 0), stop=(p == 3))
                ob = osb.tile([128, 512], F16, tag="ob", name="ob")
                nc.vector.tensor_copy(ob[:], po[:])
                nc.sync.dma_start(
                    out[128 * i:128 * (i + 1), 512 * dch:512 * (dch + 1)],
                    ob[:])

            def emit_p1(i):
                """QKV + norm/rope for token chunk i (emitted inline)."""
                pq = psum.tile([128, 512], F32, tag="pq", name="pq",
                                padded_shape=[128, 1024])
                pkv = psum.tile([128, 256], F32, tag="pkv", name="pkv",
                                padded_shape=[128, 260])
                xt = xt_tiles[i // 2]
                for dc in range(16):
                    xs = xt[:, dc, 128 * (i % 2):128 * (i % 2 + 1)]
                    nc.tensor.matmul(pq[:], xs, w_ch[dc][:, 0:512],
                                     start=(dc == 0), stop=(dc == 15))
                for dc in range(16):
                    xs = xt[:, dc, 128 * (i % 2):128 * (i % 2 + 1)]
                    nc.tensor.matmul(pkv[:], xs, w_ch[dc][:, 512:768],
                                     start=(dc == 0), stop=(dc == 15))
                qsb = p1sb.tile([128, 640], F16, tag="qsb")
                nc.scalar.copy(qsb[:, 0:512], pq[:])
                nc.scalar.copy(qsb[:, 512:640], pkv[:, 0:128])
                # v chunks straight out to vext tiles
                nc.scalar.copy(vext[0][i][:, 0:HEAD_DIM], pkv[:, 128:192])
                nc.scalar.copy(vext[1][i][:, 0:HEAD_DIM], pkv[:, 192:256])

                # --- RMSNorm (per 64) + RoPE on q (8 heads) and k (2 heads)
                sqt = p1sb.tile([128, 640], F16, tag="sqt")
                nc.scalar.square(sqt[:], qsb[:])
                ssum = p1st.tile([128, 10], F32, tag="ssum")
                nc.vector.tensor_reduce(
                    ssum[:], sqt[:].rearrange("p (h d) -> p h d", h=10),
                    axis=AX.X, op=ALU.add)
                lnv = p1st.tile([128, 10], F32, tag="std")
                nc.scalar.activation(lnv[:], ssum[:], AF.Ln,
                                     bias=teps[:], scale=1.0 / HEAD_DIM)
                rstd = p1st.tile([128, 10], F32, tag="rstd")
                nc.scalar.activation(rstd[:], lnv[:], AF.Exp, scale=-0.5)
                qn = p1sb.tile([128, 640], F16, tag="qn")
                nc.vector.tensor_mul(
                    qn[:].rearrange("p (h d) -> p h d", h=10),
                    qsb[:].rearrange("p (h d) -> p h d", h=10),
                    rstd[:].unsqueeze(2).broadcast_to([128, 10, HEAD_DIM]))
                qr = p1sb.tile([128, 640], F16, tag="qr")
                t2 = p1sb.tile([128, 640], F16, tag="t2")
                for part, nh, c_t, s_t in (("q", QH, "cq", "sq"),
                                           ("k", KVH, "ck", "sk")):
                    off = 0 if part == "q" else 512
                    qn3 = qn[:, off:off + 64 * nh].rearrange(
                        "p (h d) -> p h d", h=nh)
                    qn4 = qn[:, off:off + 64 * nh].rearrange(
                        "p (h two x) -> p h two x", h=nh, two=2)
                    cosb = tabs[c_t][:, i, :].unsqueeze(1) \
                        .broadcast_to([128, nh, HEAD_DIM])
                    sin4 = tabs[s_t][:, i, :].unsqueeze(1) \
                        .broadcast_to([128, nh, HEAD_DIM]) \
                        .rearrange("p h (two x) -> p h two x", two=2)
                    t2v = t2[:, off:off + 64 * nh].rearrange(
                        "p (h two x) -> p h two x", h=nh, two=2)
                    nc.vector.tensor_mul(t2v[:, :, 0, :], qn4[:, :, 1, :],
                                         sin4[:, :, 0, :])
                    nc.vector.tensor_mul(t2v[:, :, 1, :], qn4[:, :, 0, :],
                                         sin4[:, :, 1, :])
                    qr3 = qr[:, off:off + 64 * nh].rearrange(
                        "p (h d) -> p h d", h=nh)
                    nc.vector.tensor_mul(qr3, qn3, cosb)
                nc.vector.tensor_add(qr[:], qr[:], t2[:])
                p1_transposes(i, qr)

            def attn_J(J):
                ctxn = [cnpool.tile([128, 512], F16, tag=f"cn{jj}", name=f"cn{jj}")
                        for jj in range(4)]
                for hp in range(4):          # q heads (hp, hp+4); half hh -> kv hh
                    cbank = [psum.tile([128, 260], F32, tag="pkv", name=f"cb{w}")
                             for w in (0, 1)]
                    def epi(jj):
                        w, loc = jj // 2, jj % 2
                        cb2 = cbank[w][:, 130 * loc:130 * (loc + 1)].rearrange(
                            "p (h e) -> p h e", h=2)
                        rc = rcpool.tile([128, 2], F32, tag="rc", name="rc")
                        nc.vector.reciprocal(rc[:], cb2[:, :, 64:65].squeeze(2))
                        nc.vector.tensor_mul(
                            ctxn[jj][:, 128 * hp:128 * (hp + 1)].rearrange(
                                "p (h d) -> p h d", h=2),
                            cb2[:, :, 0:64],
                            rc[:].unsqueeze(2).broadcast_to([128, 2, HEAD_DIM]))
                        nc.sync.dma_start_transpose(
                            ctxT[hp][:, 512 * J + 128 * jj:512 * J + 128 * (jj + 1)],
                            ctxn[jj][:, 128 * hp:128 * (hp + 1)])

                    def ctx_mms(pt, jj0, c):
                        for jj in range(jj0, 4):
                            w, loc = jj // 2, jj % 2
                            for hh in (0, 1):
                                o = 130 * loc + 65 * hh
                                nc.tensor.matmul(
                                    cbank[w][:, o:o + 65],
                                    pt[:, 512 * hh + 128 * jj:512 * hh + 128 * (jj + 1)],
                                    vext[hh][c][:],
                                    start=(c == 0 and loc == 0 and hh == 0),
                                    stop=(c == 4 * J + jj and jj % 2 == 1 and hh == 1),
                                    skip_group_check=True)

                    def drain_ctx():
                        pt_, jj0_, c_ = pending.pop(0)
                        ctx_mms(pt_, jj0_, c_)
                        if c_ == 4 * J + 1:    # bank 0 (jj 0,1) is complete
                            epi(0)
                            epi(1)

                    # ctx lags scores by 2 chunks so the in-order PE queue
                    # never waits on an in-flight exp
                    pending = []
                    for c in range(4 * J + 4):
                        jj0 = max(0, c - 4 * J)
                        qlo = 128 * jj0
                        sT2 = psum.tile([128, 1024], F32, tag="pq", name="sT2")
                        for hh in (0, 1):
                            nc.tensor.matmul(
                                sT2[:, 512 * hh + qlo:512 * (hh + 1)],
                                kT[64 * hh:64 * hh + 64, 128 * c:128 * (c + 1)],
                                qT[hp][64 * hh:64 * hh + 64,
                                       512 * J + qlo:512 * (J + 1)],
                                start=True, stop=True)
                        if len(pending) >= 2:
                            drain_ctx()
                        pop_work(1)
                        pt = ptpool.tile([128, 1024], F16, tag="pt", name="pt")
                        ptv = pt[:].rearrange("p (h x) -> p h x", h=2)[:, :, 128 * jj0:512]
                        sTv = sT2[:].rearrange("p (h x) -> p h x", h=2)[:, :, 128 * jj0:512]
                        nc.scalar.activation(ptv, sTv, AF.Exp, scale=0.125)
                        if c >= 4 * J:      # diagonal: triangular mask, both heads
                            dv = pt[:].rearrange("p (h x) -> p h x", h=2)[
                                :, :, 128 * jj0:128 * (jj0 + 1)]
                            nc.vector.tensor_mul(
                                dv, dv,
                                tmask[:].unsqueeze(1).broadcast_to([128, 2, 128]))
                        pending.append((pt, jj0, c))
                    while pending:
                        drain_ctx()
                    for jj in (2, 3):
                        epi(jj)
                # defer out-proj; drained inside the next J's c-loop
                for i in range(4 * J, 4 * J + 4):
                    for dch in range(4):
                        work_q.append(lambda i=i, dch=dch: emit_po(i, dch))


            for i in range(ST):
                i2 = i // 2
                if i % 2 == 0 and i2 + 1 < ST // 2 and i2 + 1 not in xt_tiles:
                    load_xt(i2 + 1)
                emit_p1(i)
            for p in range(4):
                nc.sync.dma_start(wo_ch[p][:], woutT[128 * p:128 * (p + 1), :])
            for J in range(NJ):
                attn_J(J)
            while work_q:
                pop_work(1)

        # ================= Phase 2: attention (+ interleaved out-proj) ========
        # head pairs share one 1024-wide scores/exp tile; ctx accumulators for
        # (jj, head-half) pack into two [128, 260] psum banks per pair.
    nc.compile()
    return nc


_NC = {}


def _get_nc(repeat=1):
    if repeat not in _NC:
        _NC[repeat] = _build(repeat)
    return _NC[repeat]


_RUNNER = {}


def _get_runner(repeat=1):
    """Build (once) a jitted 8-core sharded callable around the bass program.

    Slim replica of bass2jax.run_bass_via_pjrt's multi-core path, kept
    reusable so repeated invocations skip retracing/recompilation.
    """
    if repeat in _RUNNER:
        return _RUNNER[repeat]
    import jax
    from jax.sharding import Mesh, PartitionSpec
    from jax.experimental.shard_map import shard_map
    from concourse import bass2jax
    from concourse import mybir as _mybir

    nc = _get_nc(repeat)
    bass2jax.install_neuronx_cc_hook()

    partition_name = nc.partition_id_tensor.name if nc.partition_id_tensor else None
    in_names, out_names, out_avals, zero_outs = [], [], [], []
    for alloc in nc.m.functions[0].allocations:
        if not isinstance(alloc, _mybir.MemoryLocationSet):
            continue
        name = alloc.memorylocations[0].name
        if alloc.kind == "ExternalInput":
            if name != partition_name:
                in_names.append(name)
        elif alloc.kind == "ExternalOutput":
            shape = tuple(alloc.tensor_shape)
            np_dt = _mybir.dt.np(alloc.dtype)
            out_names.append(name)
            out_avals.append(jax.core.ShapedArray(shape, np_dt))
            zero_outs.append(np.zeros(shape, np_dt))
    n_params = len(in_names)
    all_in_names = list(in_names) + list(out_names)
    if partition_name is not None:
        all_in_names.append(partition_name)

    def _body(*args):
        operands = list(args)
        if partition_name is not None:
            operands.append(bass2jax.partition_id_tensor())
        outs = bass2jax._bass_exec_p.bind(
            *operands,
            out_avals=tuple(out_avals),
            in_names=tuple(all_in_names),
            out_names=tuple(out_names),
            lowering_input_output_aliases=(),
            sim_require_finite=True,
            sim_require_nnan=True,
            nc=nc,
        )
        return tuple(outs)

    devices = jax.devices()[:N_CORES]
    mesh = Mesh(np.asarray(devices), ("core",))
    in_specs = (PartitionSpec("core"),) * (n_params + len(out_names))
    out_specs = (PartitionSpec("core"),) * len(out_names)
    sharded = jax.jit(shard_map(_body, mesh=mesh, in_specs=in_specs,
                                out_specs=out_specs, check_rep=False),
                      keep_unused=True)

    concat_zeros = [np.zeros((N_CORES * z.shape[0], *z.shape[1:]), z.dtype)
                    for z in zero_outs]

    _dev_cache = {}

    def run(in_maps, iters=1, time_list=None, fetch=True):
        import time as _time
        from jax.sharding import NamedSharding
        shard = NamedSharding(mesh, PartitionSpec("core"))
        key = id(in_maps)
        if key not in _dev_cache:
            per_core = [[np.asarray(m[nm]) for nm in in_names] for m in in_maps]
            concat_in = [np.concatenate([per_core[c][i] for c in range(N_CORES)],
                                        axis=0) for i in range(n_params)]
            dev_in = [jax.device_put(a, shard) for a in concat_in]
            dev_zero = [jax.device_put(z, shard) for z in concat_zeros]
            jax.block_until_ready(dev_in)
            _dev_cache.clear()
            _dev_cache[key] = (dev_in, dev_zero)
        dev_in, dev_zero = _dev_cache[key]
        out_arrs = None
        if iters <= 1:
            out_arrs = sharded(*dev_in, *dev_zero)
            jax.block_until_ready(out_arrs)
        else:
            # async batch: submit all, block once; caller computes slope
            sharded(*dev_in, *dev_zero)  # warm
            t0 = _time.perf_counter()
            for _ in range(iters):
                out_arrs = sharded(*dev_in, *dev_zero)
            jax.block_until_ready(out_arrs)
            if time_list is not None:
                time_list.append(_time.perf_counter() - t0)
        if not fetch:
            del out_arrs
            return None
        return [
            {nm: np.asarray(out_arrs[i]).reshape(N_CORES, *out_avals[i].shape)[c]
             for i, nm in enumerate(out_names)}
            for c in range(N_CORES)
        ]

    _RUNNER[repeat] = run
    return run


def _host_tables(q_gamma, k_gamma):
    pos = np.arange(S, dtype=np.float32)
    inv = 1.0 / (ROPE_FREQ ** (np.arange(0, HEAD_DIM, 2, dtype=np.float32)
                               / HEAD_DIM))
    fr = pos[:, None] * inv[None, :]
    emb = np.concatenate([fr, fr], axis=-1)
    cos = np.cos(emb).astype(np.float32)
    sin = np.sin(emb).astype(np.float32)
    outs = []
    for gamma in (q_gamma, k_gamma):
        g = gamma.astype(np.float32)
        cos_g = cos * g[None, :]
        sin_eff = np.concatenate([-sin[:, :32] * g[None, 32:],
                                  sin[:, 32:] * g[None, :32]], axis=-1)
        outs += [cos_g.astype(np.float16), sin_eff.astype(np.float16)]
    return outs  # cos_q, sin_q, cos_k, sin_k


def _make_in_maps(x, w_qkv, w_out, q_gamma, k_gamma):
    cos_q, sin_q, cos_k, sin_k = _host_tables(q_gamma, k_gamma)
    mask = (np.arange(128)[None, :] >= np.arange(128)[:, None]).astype(np.float16)
    # q heads packed in pairs (p, p+4) so a pair's partition halves map to
    # kv heads (0, 1) = kT's natural layout
    qperm = [0, 4, 1, 5, 2, 6, 3, 7]

    in_maps = []
    for core in range(N_CORES):
        b, g = core // 4, core % 4
        xT = np.ascontiguousarray(x[b].T).astype(np.float16)
        wq = w_qkv[512 * g:512 * (g + 1)]                      # 8 q heads
        wq = wq.reshape(8, HEAD_DIM, D)[qperm].reshape(512, D)
        wk = w_qkv[2048 + 128 * g:2048 + 128 * (g + 1)]        # 2 k heads
        wv = w_qkv[2560 + 128 * g:2560 + 128 * (g + 1)]        # 2 v heads
        wqkvT = np.ascontiguousarray(
            np.concatenate([wq, wk, wv], axis=0).T).astype(np.float16)
        woutT = w_out[:, 512 * g:512 * (g + 1)].T              # [512, D]
        woutT = np.ascontiguousarray(
            woutT.reshape(8, HEAD_DIM, D)[qperm].reshape(512, D)
        ).astype(np.float16)
        in_maps.append({
            "xT": xT, "wqkvT": wqkvT, "woutT": woutT,
            "cos_q": cos_q, "sin_q": sin_q, "cos_k": cos_k, "sin_k": sin_k,
            "mask_i": mask,
        })
    return in_maps


def kernel(x, w_qkv, w_out, q_gamma, k_gamma):
    x = np.asarray(x)
    w_qkv = np.asarray(w_qkv)
    w_out = np.asarray(w_out)
    q_gamma = np.asarray(q_gamma)
    k_gamma = np.asarray(k_gamma)
    in_maps = _make_in_maps(x, w_qkv, w_out, q_gamma, k_gamma)
    results = _get_runner()(in_maps)
    parts = [results[c]["out"] for c in range(N_CORES)]
    out = np.empty((B, S, D), dtype=np.float32)
    for b in range(B):
        acc = parts[4 * b].astype(np.float32)
        for g in range(1, 4):
            acc += parts[4 * b + g].astype(np.float32)
        out[b] = acc
    return out

